# revision 65
# baseline (speedup 1.0000x reference)
"""BitNetAttention Trainium2 kernel — 8-core SPMD, query-sharded, collective-free.

Per core c: batch b = c//4, query rows 512*(c%4)..+512. The host hands each
core its batch's FULL hidden states rotated so the core's own 512 query rows
sit at rows 0:511 (softmax is key-permutation invariant, so rotating the key
axis is harmless). Each core int4-quantizes all 2048 rows (redundant across
the 4-core batch group — cheaper than any collective), computes q for its 512
queries and k/v for all 2048 keys with exact fp8e4m3 DoubleRow matmuls
(int4 values and ternary weights are exact in fp8); each key slice's
projections are emitted in 4 pieces interleaved with the next 4 tiles' quant
chains so no engine queue blocks. Rounding to int4/int8 levels uses the fp32
+3*2^22 magic constant (exact RNE, matches np.round; hardware rounds per-op
so fp16 magic is NOT exact), with the clip split across DVE (max) and Pool
(min, SBUF-only — GPSIMD cannot touch PSUM). Attention: scoresT layout in
f32r, exp on ACT (the saturated engine: ~133us floor), PV matmul with a 65th
all-ones lhsT column yielding the softmax denominator Z exactly. Tail per
query-half: int8-quant + top-50% sparsify via real-valued bisection (7
halvings of [-0.5,127], counts via DVE tensor_scalar is_le with op1=add
accum, final mask ab > hi-1); half A's tail overlaps half B's attention.
O-projection per half with host-relayout Wo ([ft,p,ct,m] contiguous bf16
rows) streamed at use, transposes+copies ahead of ct-ordered matmuls into 4
single-ft PSUM accumulators, per-token scale folded onto output columns;
half A's o-proj overlaps half B's tail. No collectives. Host reassembles
transposed shards.
"""
import sys
import math

sys.path.insert(0, "/opt/trn_rl_repo")

import numpy as np

B, S, H, NH = 2, 2048, 1024, 16
HD = H // NH          # 64
SHARD = 512           # query rows per core
NCORES = 8
SQRT7 = math.sqrt(7.0)
MAGIC = float(np.float32(3 * 2**22))   # fp32 RNE magic (used on tiny tiles)
MAGIC16 = 1536.0                       # 1.5 * 2^10: fp16 RNE magic

_cache = {}


def _build():
    import concourse.bass as bass
    import concourse.bacc as bacc
    import concourse.mybir as mybir
    from concourse.tile import TileContext
    from concourse.masks import make_identity

    dt = mybir.dt
    Alu = mybir.AluOpType
    Act = mybir.ActivationFunctionType
    X = mybir.AxisListType.X
    DR = mybir.MatmulPerfMode.DoubleRow

    HT_ = H // 128

    nc = bacc.Bacc("TRN2", target_bir_lowering=False, debug=False,
                   num_devices=NCORES)

    hs_in = nc.dram_tensor("hs", [S, H], dt.float32, kind="ExternalInput")
    wq8_in = nc.dram_tensor("wq8", [H, H], dt.float8e4, kind="ExternalInput")
    wk8_in = nc.dram_tensor("wk8", [H, H], dt.float8e4, kind="ExternalInput")
    wv8_in = nc.dram_tensor("wv8", [H, H], dt.float8e4, kind="ExternalInput")
    woH_in = nc.dram_tensor("woH", [HT_, 128, HT_, 128], dt.bfloat16,
                            kind="ExternalInput")
    cst_in = nc.dram_tensor("cst", [8], dt.float32, kind="ExternalInput")
    outT_out = nc.dram_tensor("outT", [H, SHARD], dt.float32, kind="ExternalOutput")

    brow = nc.dram_tensor("brow", [S], dt.float32)
    srow = nc.dram_tensor("srow", [SHARD], dt.float16)
    qTd = nc.dram_tensor("qTd", [H, SHARD], dt.float32r)

    NT = S // 128       # 16 s-tiles (all tokens)
    QT = SHARD // 128   # 4 q-tiles (own queries)
    HT = H // 128       # 8 h/f/c-tiles
    KT = S // 128       # 16 k-tiles

    IS7 = float(np.float32(1.0 / (H * SQRT7)))      # 1/(H*sqrt7)
    ES7 = float(np.float32(1e-5 / SQRT7))           # 1e-5/sqrt7

    with TileContext(nc) as tc:
        with tc.tile_pool(name="base", bufs=1) as bp, \
             tc.tile_pool(name="work", bufs=2) as wp, \
             tc.tile_pool(name="mmps", bufs=2, space="PSUM") as pmm:

            ident = bp.tile([128, 128], dt.float32)
            make_identity(nc, ident[:])
            identb = bp.tile([128, 128], dt.bfloat16)
            nc.vector.tensor_copy(identb[:], ident[:])
            identh = bp.tile([128, 128], dt.float16)
            nc.vector.tensor_copy(identh[:], ident[:])
            ones_row = bp.tile([1, 128], dt.float32)
            nc.vector.memset(ones_row[:], 1.0)
            ones16 = bp.tile([1, 128], dt.float16)
            nc.vector.memset(ones16[:], 1.0)

            cst_sb = bp.tile([1, 8], dt.float32)
            nc.sync.dma_start(out=cst_sb[:], in_=cst_in[None, :])
            ps_c = pmm.tile([128, 512], dt.float32, tag="mm")
            nc.tensor.matmul(ps_c[:, 0:8], ones_row[:], cst_sb[:], start=True, stop=True)
            cst_bc = bp.tile([128, 8], dt.float32)
            nc.vector.tensor_copy(cst_bc[:], ps_c[:, 0:8])
            AQ8 = cst_bc[:, 0:1]
            AK = cst_bc[:, 1:2]
            AV = cst_bc[:, 2:3]
            AO127 = cst_bc[:, 3:4]

            av_cols = bp.tile([128, NT], dt.float32)

            with tc.tile_pool(name="kv", bufs=1) as kvp:
                kT = kvp.tile([128, HT, S], dt.float32r)
                vres = kvp.tile([128, KT, NH, HD + 1], dt.float32r)

                # ====== merged phase 1-3: quant + projections, pipelined ====
                with tc.tile_pool(name="xq", bufs=1) as xp, \
                     tc.tile_pool(name="wts", bufs=1) as wtp, \
                     tc.tile_pool(name="scl", bufs=1) as sp, \
                     tc.tile_pool(name="qtmp", bufs=1) as qp, \
                     tc.tile_pool(name="prps", bufs=5, space="PSUM") as prps, \
                     tc.tile_pool(name="tps8", bufs=1, space="PSUM") as tpp:
                    xqT8 = xp.tile([128, HT, S], dt.float8e4)
                    wq8 = wtp.tile([128, HT, H], dt.float8e4)
                    wk8 = wtp.tile([128, HT, H], dt.float8e4)
                    wv8 = wtp.tile([128, HT, H], dt.float8e4)
                    aq_bc = sp.tile([128, SHARD], dt.float32)
                    ak_tiles = {}

                    def emit_piece(ks, p):
                        if p == 0:
                            # per-slice scale row: keys ks*512..+512
                            beta_row = wp.tile([1, 512], dt.float32, tag="brw",
                                               bufs=1)
                            nc.sync.dma_start(
                                out=beta_row[:],
                                in_=brow[None, ks * 512:(ks + 1) * 512])
                            ps_a = pmm.tile([128, 512], dt.float32, tag="mm")
                            nc.tensor.matmul(ps_a[:], ones_row[:], beta_row[:],
                                             start=True, stop=True)
                            ak_sl = sp.tile([128, 512], dt.float32,
                                            tag="aksl", bufs=2, name=f"ak{ks}")
                            ak_tiles[ks] = ak_sl
                            nc.vector.tensor_scalar(
                                out=ak_sl[:], in0=ps_a[:], scalar1=AK,
                                scalar2=None, op0=Alu.mult)
                            if ks == 0:
                                nc.vector.tensor_scalar(out=aq_bc[:], in0=ps_a[:],
                                                        scalar1=AQ8, scalar2=None,
                                                        op0=Alu.mult)
                        if ks == 0 and p >= 2:
                            # q projection (own 512 rows), 4 ft per piece
                            for ft in range(4 * (p - 2), 4 * (p - 2) + 4):
                                ps = prps.tile([128, 512], dt.float32, tag="pj")
                                for hp in range(4):
                                    nc.tensor.matmul(ps[:],
                                                     wq8[:, 2 * hp:2 * hp + 2, ft * 128:(ft + 1) * 128],
                                                     xqT8[:, 2 * hp:2 * hp + 2, 0:SHARD],
                                                     start=(hp == 0), stop=(hp == 3),
                                                     perf_mode=DR)
                                qsl = qp.tile([128, SHARD], dt.float32r, tag="qsl", bufs=1)
                                nc.vector.tensor_tensor(out=qsl[:], in0=ps[:],
                                                        in1=aq_bc[:], op=Alu.mult)
                                nc.sync.dma_start(
                                    out=qTd[ft * 128:(ft + 1) * 128, :], in_=qsl[:])
                        # k projection, 2 ft per piece
                        for ft in (2 * p, 2 * p + 1):
                            ps = prps.tile([128, 512], dt.float32, tag="pj")
                            for hp in range(4):
                                nc.tensor.matmul(ps[:],
                                                 wk8[:, 2 * hp:2 * hp + 2, ft * 128:(ft + 1) * 128],
                                                 xqT8[:, 2 * hp:2 * hp + 2, ks * 512:(ks + 1) * 512],
                                                 start=(hp == 0), stop=(hp == 3),
                                                 perf_mode=DR)
                            nc.vector.tensor_tensor(
                                out=kT[:, ft, ks * 512:(ks + 1) * 512], in0=ps[:],
                                in1=ak_tiles[ks][:], op=Alu.mult)
                        # v projection: k-tiles spread over pieces 1-3
                        for dkt in [[], [0], [1, 2], [3]][p]:
                            kt = 4 * ks + dkt
                            for fc in range(2):
                                ps = prps.tile([128, 512], dt.float32, tag="pj")
                                for hp in range(4):
                                    nc.tensor.matmul(ps[:],
                                                     xqT8[:, 2 * hp:2 * hp + 2, kt * 128:(kt + 1) * 128],
                                                     wv8[:, 2 * hp:2 * hp + 2, fc * 512:(fc + 1) * 512],
                                                     start=(hp == 0), stop=(hp == 3),
                                                     perf_mode=DR)
                                nc.scalar.activation(
                                    vres[:, kt, fc * 8:(fc + 1) * 8, 0:HD],
                                    ps[:].rearrange("p (h d) -> p h d", d=HD),
                                    Act.Copy, scale=av_cols[:, kt:kt + 1])

                    for i in range(NT):
                        hst = qp.tile([128, H], dt.float32, tag="hs", bufs=3)
                        nc.sync.dma_start(out=hst[:], in_=hs_in[i * 128:(i + 1) * 128, :])
                        # weight loads deferred behind the first hidden-state
                        # tiles: not needed until the i==3 projection burst
                        if i == 1:
                            nc.sync.dma_start(out=wk8[:], in_=wk8_in.rearrange("(a p) f -> p a f", p=128))
                        elif i == 3:
                            nc.sync.dma_start(out=wv8[:], in_=wv8_in.rearrange("(a p) f -> p a f", p=128))
                        elif i == 5:
                            nc.sync.dma_start(out=wq8[:], in_=wq8_in.rearrange("(a p) f -> p a f", p=128))
                        absj = qp.tile([128, H], dt.float8e4, tag="absj", bufs=1)
                        ssum = wp.tile([128, 1], dt.float32, tag="ssum", bufs=6)
                        nc.scalar.activation(absj[:], hst[:], Act.Abs,
                                             accum_out=ssum[:])
                        beta = wp.tile([128, 1], dt.float32, tag="beta", bufs=6)
                        nc.vector.tensor_scalar(out=beta[:], in0=ssum[:],
                                                scalar1=float(np.float32(1.0 / H)),
                                                scalar2=None, op0=Alu.mult)
                        nc.sync.dma_start(out=brow[i * 128:(i + 1) * 128], in_=beta[:, 0])
                        nc.vector.tensor_scalar(out=av_cols[:, i:i + 1], in0=beta[:],
                                                scalar1=AV, scalar2=None, op0=Alu.mult)
                        # r2s7 = sqrt7 / (beta + 1e-5) in one recip:
                        # denom2 = ssum/(H*s7) + 1e-5/s7
                        denom2 = wp.tile([128, 1], dt.float32, tag="dn2", bufs=6)
                        nc.vector.tensor_scalar(out=denom2[:], in0=ssum[:],
                                                scalar1=IS7, scalar2=ES7,
                                                op0=Alu.mult, op1=Alu.add)
                        r2s7 = wp.tile([128, 1], dt.float32, tag="r2s7", bufs=6)
                        nc.vector.reciprocal(r2s7[:], denom2[:])
                        # exact RNE to integer via the fp32 magic constant
                        # (single fp32 rounding, matches np.round); clip after
                        # (max on DVE, min on the idle Pool engine)
                        y1 = qp.tile([128, H], dt.float32, tag="y1", bufs=1)
                        nc.vector.tensor_scalar(out=y1[:], in0=hst[:],
                                                scalar1=r2s7[:],
                                                scalar2=MAGIC,
                                                op0=Alu.mult, op1=Alu.add)
                        y2 = qp.tile([128, H], dt.float16, tag="y2", bufs=2)
                        nc.vector.tensor_scalar(out=y2[:], in0=y1[:],
                                                scalar1=MAGIC,
                                                scalar2=float(np.float32(-8.0)),
                                                op0=Alu.subtract, op1=Alu.max)
                        nc.gpsimd.tensor_scalar(out=y2[:], in0=y2[:],
                                                scalar1=float(np.float32(7.0)),
                                                scalar2=None, op0=Alu.min)
                        tp8 = tpp.tile([128, H], dt.float16, tag="tp8")
                        for jt in range(HT):
                            nc.tensor.transpose(tp8[:, jt * 128:(jt + 1) * 128],
                                                y2[:, jt * 128:(jt + 1) * 128],
                                                identh[:])
                        # PSUM->SBUF convert to fp8 (GPSIMD cannot read PSUM)
                        if i % 2 == 0:
                            nc.scalar.activation(
                                xqT8[:, :, i * 128:(i + 1) * 128],
                                tp8[:].rearrange("p (a q) -> p a q", q=128),
                                Act.Copy)
                        else:
                            nc.vector.tensor_copy(
                                xqT8[:, :, i * 128:(i + 1) * 128],
                                tp8[:].rearrange("p (a q) -> p a q", q=128))

                        # projection work of slice ks is emitted in 4
                        # pieces interleaved with the NEXT 4 tiles' quant
                        # chains, so no engine queue blocks long on PE bursts
                        if i >= 4:
                            emit_piece((i // 4) - 1, i % 4)

                    for p in range(4):
                        emit_piece(3, p)

                ones_f = wp.tile([128, NH], dt.float32, tag="onesf")
                nc.vector.memset(ones_f[:], 1.0)
                ones_b = wp.tile([128, NH], dt.float32r, tag="onesb")
                nc.vector.tensor_copy(ones_b[:], ones_f[:])
                for t in range(KT):
                    nc.vector.tensor_copy(
                        vres[:, t, :, HD:HD + 1],
                        ones_b.rearrange("p (h o) -> p h o", o=1))

                # ===== phase 4+5: attention by query-halves; the tail and
                # o-projection of half A (DVE/ACT/PE mix) run concurrently
                # with attention of half B ===================================
                QW = 256
                with tc.tile_pool(name="tailp", bufs=1) as tlp:
                    nm = tlp.tile([128, QT, H], dt.bfloat16)
                    ctx = tlp.tile([128, QT, H], dt.float32)

                    def emit_tail_half(qh):
                        sts = (2 * qh, 2 * qh + 1)
                        nbs2, abs2, junks2 = [], [], []
                        for s2, st in enumerate(sts):
                            cx = ctx[:, st, :]
                            gmax = wp.tile([128, 1], dt.float32, tag=f"gm{s2}")
                            nc.vector.tensor_reduce(gmax[:], cx, axis=X, op=Alu.max,
                                                    apply_absolute_value=True)
                            gmax = gmax[:]
                            gd = wp.tile([128, 1], dt.float32, tag=f"gd{s2}")
                            nc.vector.tensor_scalar(out=gd[:], in0=gmax,
                                                    scalar1=float(np.float32(1e-5)),
                                                    scalar2=None, op0=Alu.add)
                            rg = wp.tile([128, 1], dt.float32, tag=f"rg{s2}")
                            nc.vector.reciprocal(rg[:], gd[:])
                            rg127 = wp.tile([128, 1], dt.float32, tag=f"rh{s2}")
                            nc.vector.tensor_scalar(out=rg127[:], in0=rg[:],
                                                    scalar1=float(np.float32(127.0)),
                                                    scalar2=None, op0=Alu.mult)
                            sc = wp.tile([128, 1], dt.float16, tag=f"sc{s2}")
                            nc.vector.tensor_scalar(out=sc[:], in0=gmax,
                                                    scalar1=AO127,
                                                    scalar2=None, op0=Alu.mult)
                            nc.sync.dma_start(out=srow[st * 128:(st + 1) * 128],
                                              in_=sc[:, 0])
                            # y = cx*rg*127 + 1536 -> fp16 RNE to int8 level
                            y = tlp.tile([128, H], dt.float16, tag="y", bufs=2)
                            nc.vector.tensor_scalar(out=y[:], in0=cx,
                                                    scalar1=rg127[:],
                                                    scalar2=MAGIC16,
                                                    op0=Alu.mult, op1=Alu.add)
                            nb = tlp.tile([128, H], dt.bfloat16, tag="nb", bufs=2)
                            nc.vector.tensor_scalar(out=nb[:], in0=y[:],
                                                    scalar1=MAGIC16, scalar2=None,
                                                    op0=Alu.subtract)
                            ab = tlp.tile([128, H], dt.bfloat16, tag="ab", bufs=2)
                            nc.vector.scalar_tensor_tensor(out=ab[:], in0=nb[:],
                                                           scalar=-1.0, in1=nb[:],
                                                           op0=Alu.mult, op1=Alu.max)
                            junk = tlp.tile([128, H], dt.bfloat16, tag="junk", bufs=2)
                            nbs2.append(nb); abs2.append(ab); junks2.append(junk)
                        # Real-valued bisection for the 512th-smallest |level|:
                        # invariant cnt_le(lo) < 512 <= cnt_le(hi); levels are
                        # integers in [0,127], so 7 halvings of [-0.5, 127]
                        # leave hi-lo < 1 and the threshold t* = the unique
                        # integer in (lo, hi]. Mask keeps ab > hi-1 == ab >= t*.
                        lo2 = wp.tile([128, 2], dt.float32, tag="lo2")
                        hi2 = wp.tile([128, 2], dt.float32, tag="hi2")
                        mid2 = wp.tile([128, 2], dt.float32, tag="mid2")
                        cnt2 = wp.tile([128, 2], dt.float32, tag="cnt2")
                        tk2 = wp.tile([128, 2], dt.uint32, tag="tk2")
                        nc.vector.memset(lo2[:], -0.5)
                        nc.vector.memset(hi2[:], 127.0)
                        for it in range(7):
                            nc.vector.tensor_tensor(out=mid2[:], in0=lo2[:],
                                                    in1=hi2[:], op=Alu.add)
                            nc.vector.tensor_scalar(out=mid2[:], in0=mid2[:],
                                                    scalar1=float(np.float32(0.5)),
                                                    scalar2=None, op0=Alu.mult)
                            # per-slice count on DVE: #(ab <= mid); tensor_scalar
                            # +accum gets the 4x 16-bit DVE mode (~327ns)
                            for s2 in range(2):
                                nc.vector.tensor_scalar(
                                    out=junks2[s2][:], in0=abs2[s2][:],
                                    scalar1=mid2[:, s2:s2 + 1],
                                    scalar2=None,
                                    op0=Alu.is_le, op1=Alu.add,
                                    accum_out=cnt2[:, s2:s2 + 1])
                            nc.vector.tensor_scalar(out=tk2[:], in0=cnt2[:],
                                                    scalar1=float(np.float32(512.0)),
                                                    scalar2=None, op0=Alu.is_ge)
                            nc.vector.copy_predicated(hi2[:], tk2[:], mid2[:])
                            nc.vector.tensor_scalar(out=tk2[:], in0=cnt2[:],
                                                    scalar1=float(np.float32(512.0)),
                                                    scalar2=None, op0=Alu.is_lt)
                            nc.vector.copy_predicated(lo2[:], tk2[:], mid2[:])
                        th2 = wp.tile([128, 2], dt.float32, tag="th2")
                        nc.vector.tensor_scalar(out=th2[:], in0=hi2[:],
                                                scalar1=float(np.float32(-1.0)),
                                                scalar2=None, op0=Alu.add)
                        for s2, st in enumerate(sts):
                            nc.vector.scalar_tensor_tensor(
                                out=nm[:, st, :], in0=abs2[s2][:],
                                scalar=th2[:, s2:s2 + 1], in1=nbs2[s2][:],
                                op0=Alu.is_gt, op1=Alu.mult)

                    with tc.tile_pool(name="scps", bufs=2, space="PSUM") as psc, \
                         tc.tile_pool(name="ctxps", bufs=2, space="PSUM") as pcx, \
                         tc.tile_pool(name="probs", bufs=3) as prp, \
                         tc.tile_pool(name="cwork", bufs=2) as cwp:
                        for qh in range(2):
                            qlo = qh * QW
                            for pr in range(NH // 2):
                                hA, hB = 2 * pr, 2 * pr + 1
                                qTs = cwp.tile([128, QW], dt.float32r, tag="qts")
                                nc.sync.dma_start(
                                    out=qTs[:],
                                    in_=qTd[pr * 128:(pr + 1) * 128, qlo:qlo + QW])
                                pcA = pcx.tile([HD + 1, QW], dt.float32, tag="ctx")
                                pcB = pcx.tile([HD + 1, QW], dt.float32, tag="ctx")
                                for g in range(KT // 4):
                                    psA = psc.tile([128, 1024], dt.float32, tag="sc")
                                    psB = psc.tile([128, 1024], dt.float32, tag="sc")
                                    for gi in range(4):
                                        t = 4 * g + gi
                                        ksl = kT[:, pr, t * 128:(t + 1) * 128]
                                        nc.tensor.matmul(psA[:, gi * QW:(gi + 1) * QW],
                                                         ksl[0:64, :], qTs[0:64, :],
                                                         start=True, stop=True,
                                                         tile_position=(0, 0))
                                        nc.tensor.matmul(psB[:, gi * QW:(gi + 1) * QW],
                                                         ksl[64:128, :], qTs[64:128, :],
                                                         start=True, stop=True,
                                                         tile_position=(64, 0))
                                    pbA = prp.tile([128, 1024], dt.float32r, tag="pb")
                                    pbB = prp.tile([128, 1024], dt.float32r, tag="pb")
                                    nc.scalar.activation(pbA[:], psA[:], Act.Exp)
                                    nc.scalar.activation(pbB[:], psB[:], Act.Exp)
                                    for gi in range(4):
                                        t = 4 * g + gi
                                        nc.tensor.matmul(pcA[:], vres[:, t, hA, :],
                                                         pbA[:, gi * QW:(gi + 1) * QW],
                                                         start=(t == 0), stop=(t == KT - 1))
                                        nc.tensor.matmul(pcB[:], vres[:, t, hB, :],
                                                         pbB[:, gi * QW:(gi + 1) * QW],
                                                         start=(t == 0), stop=(t == KT - 1))
                                csbA = cwp.tile([HD + 1, QW], dt.float32, tag="csbA")
                                nc.vector.tensor_copy(csbA[:], pcA[:])
                                csbB = cwp.tile([HD + 1, QW], dt.float32, tag="csbB")
                                nc.vector.tensor_copy(csbB[:], pcB[:])
                                for st2 in range(2):
                                    st = 2 * qh + st2
                                    for hx, csb in ((0, csbA), (1, csbB)):
                                        pt = pmm.tile([128, 512], dt.float32, tag="mm")
                                        nc.tensor.transpose(
                                            pt[:, 0:HD + 1],
                                            csb[:, st2 * 128:(st2 + 1) * 128],
                                            ident[0:HD + 1, 0:HD + 1])
                                        rz = wp.tile([128, 1], dt.float32)
                                        nc.vector.reciprocal(rz[:], pt[:, HD:HD + 1])
                                        nc.vector.tensor_scalar(
                                            out=ctx[:, st, (hA + hx) * HD:(hA + hx + 1) * HD],
                                            in0=pt[:, 0:HD], scalar1=rz[:],
                                            scalar2=None, op0=Alu.mult)
                            # tail of half A overlaps half B's attention;
                            # half B's tail is emitted in phase 6 after half
                            # A's o-projection so A's output DMAs go early
                            if qh == 0:
                                emit_tail_half(0)

                    # ===== phase 6: o-projection per query-half. Half A
                    # depends only on tail A (long done), so its PE work
                    # overlaps tail B's DVE/ACT bisection. ==================
                    with tc.tile_pool(name="tpbp", bufs=2, space="PSUM") as tbp, \
                         tc.tile_pool(name="opps", bufs=4, space="PSUM") as opp, \
                         tc.tile_pool(name="oph", bufs=1) as oph:
                        def emit_oproj_half(qh):
                            # per-token output scale, broadcast across rows
                            sc_row = wp.tile([1, QW], dt.float16, tag="scrow", bufs=1)
                            nc.sync.dma_start(out=sc_row[:],
                                              in_=srow[None, qh * QW:(qh + 1) * QW])
                            ps_s = pmm.tile([128, 512], dt.float32, tag="mm")
                            nc.tensor.matmul(ps_s[:, 0:QW], ones16[:], sc_row[:],
                                             start=True, stop=True)
                            sc_bc = oph.tile([128, QW], dt.float32, tag="scbc", bufs=2)
                            nc.vector.tensor_copy(sc_bc[:], ps_s[:, 0:QW])
                            rhsT = oph.tile([128, HT, QW], dt.bfloat16, tag="rhsT",
                                            bufs=1)
                            # all transposes+copies first, then matmuls, so PE
                            # is never queue-blocked behind a copy; two waves
                            # of 4 ft so each accumulation chain owns a bank
                            for ct in range(HT):
                                tpb = tbp.tile([128, QW], dt.bfloat16, tag="tpb")
                                for st2, st in enumerate((2 * qh, 2 * qh + 1)):
                                    nc.tensor.transpose(
                                        tpb[:, st2 * 128:(st2 + 1) * 128],
                                        nm[:, st, ct * 128:(ct + 1) * 128],
                                        identb[:])
                                nc.scalar.activation(rhsT[:, ct, :], tpb[:],
                                                     Act.Copy)
                            for wv in range(2):
                                pss = [opp.tile([128, QW], dt.float32, tag="op",
                                                name=f"op{qh}_{wv}_{fp}")
                                       for fp in range(4)]
                                wsls = []
                                for fp in range(4):
                                    ft = 4 * wv + fp
                                    wsl = oph.tile([128, HT, 128], dt.bfloat16,
                                                   tag="wsl", bufs=7)
                                    nc.sync.dma_start(out=wsl[:], in_=woH_in[ft])
                                    wsls.append(wsl)
                                for ct in range(HT):
                                    for fp in range(4):
                                        nc.tensor.matmul(pss[fp][:],
                                                         wsls[fp][:, ct, :],
                                                         rhsT[:, ct, :], start=(ct == 0),
                                                         stop=(ct == HT - 1))
                                for fp in range(4):
                                    ft = 4 * wv + fp
                                    ot = oph.tile([128, QW], dt.float32, tag="ot", bufs=4)
                                    nc.vector.tensor_tensor(out=ot[:], in0=pss[fp][:],
                                                            in1=sc_bc[:], op=Alu.mult)
                                    nc.sync.dma_start(
                                        out=outT_out[ft * 128:(ft + 1) * 128,
                                                     qh * QW:(qh + 1) * QW],
                                        in_=ot[:])

                        emit_oproj_half(0)
                        emit_tail_half(1)
                        emit_oproj_half(1)

    nc.compile()
    return nc


def kernel(hidden_states, Wq, Wk, Wv, Wo, sq, sk, sv, so):
    import jax
    import jax.numpy as jnp
    from concourse.bass_utils import run_bass_kernel_spmd
    import ml_dtypes

    cpu = jax.devices("cpu")[0]

    def wquant(W, s):
        with jax.default_device(cpu):
            W32 = np.asarray(W, np.float32)
            w_mean = jnp.mean(jnp.abs(jnp.asarray(W32)))
            w_q = jnp.clip(jnp.round(jnp.asarray(W32) / (w_mean + 1e-5)), -1.0, 1.0)
            return np.asarray(w_q, np.float32), np.float32(np.float32(w_mean) * np.float32(s))

    hidden_states = np.ascontiguousarray(np.asarray(hidden_states, np.float32))
    wq_q, aq = wquant(Wq, np.asarray(sq).reshape(-1)[0])
    wk_q, ak = wquant(Wk, np.asarray(sk).reshape(-1)[0])
    wv_q, av = wquant(Wv, np.asarray(sv).reshape(-1)[0])
    wo_q, ao = wquant(Wo, np.asarray(so).reshape(-1)[0])

    wq8 = np.ascontiguousarray(wq_q.T).astype(ml_dtypes.float8_e4m3)
    wk8 = np.ascontiguousarray(wk_q.T).astype(ml_dtypes.float8_e4m3)
    wv8 = np.ascontiguousarray(wv_q.T).astype(ml_dtypes.float8_e4m3)
    woT_mat = np.ascontiguousarray(wo_q.T)
    woH = np.ascontiguousarray(
        woT_mat.reshape(H // 128, 128, H // 128, 128).transpose(2, 1, 0, 3)
    ).astype(ml_dtypes.bfloat16)

    cst = np.zeros(8, np.float32)
    cst[0] = np.float32(aq / np.float32(math.sqrt(HD)))
    cst[1] = ak
    cst[2] = av
    cst[3] = np.float32(ao / np.float32(127.0))

    if "nc" not in _cache:
        _cache["nc"] = _build()
    nc = _cache["nc"]

    in_maps = []
    for c in range(NCORES):
        b, j = c // 4, c % 4
        hs_rot = np.ascontiguousarray(np.roll(hidden_states[b], -j * SHARD, axis=0))
        in_maps.append({
            "hs": hs_rot,
            "wq8": wq8, "wk8": wk8, "wv8": wv8, "woH": woH, "cst": cst,
        })

    _cache["last_in_maps"] = in_maps
    res = run_bass_kernel_spmd(nc, in_maps, list(range(NCORES)))
    _cache["last_res"] = res
    out = np.empty((B, S, H), np.float32)
    for c in range(NCORES):
        b, j = c // 4, c % 4
        out[b, j * SHARD:(j + 1) * SHARD, :] = res.results[c]["outT"].T
    return out


# revision 66
# speedup vs baseline: 1.0034x; 1.0034x over previous
"""BitNetAttention Trainium2 kernel — 8-core SPMD, query-sharded, collective-free.

Per core c: batch b = c//4, query rows 512*(c%4)..+512. The host hands each
core its batch's FULL hidden states rotated so the core's own 512 query rows
sit at rows 0:511 (softmax is key-permutation invariant, so rotating the key
axis is harmless). Each core int4-quantizes all 2048 rows (redundant across
the 4-core batch group — cheaper than any collective), computes q for its 512
queries and k/v for all 2048 keys with exact fp8e4m3 DoubleRow matmuls
(int4 values and ternary weights are exact in fp8); each key slice's
projections are emitted in 4 pieces interleaved with the next 4 tiles' quant
chains so no engine queue blocks. Rounding to int4/int8 levels uses the fp32
+3*2^22 magic constant (exact RNE, matches np.round; hardware rounds per-op
so fp16 magic is NOT exact), with the clip split across DVE (max) and Pool
(min, SBUF-only — GPSIMD cannot touch PSUM). Attention: scoresT layout in
f32r, exp on ACT (the saturated engine: ~133us floor), PV matmul with a 65th
all-ones lhsT column yielding the softmax denominator Z exactly. Tail per
query-half: int8-quant + top-50% sparsify via real-valued bisection (7
halvings of [-0.5,127], counts via DVE tensor_scalar is_le with op1=add
accum, final mask ab > hi-1); half A's tail overlaps half B's attention.
O-projection per half with host-relayout Wo ([ft,p,ct,m] contiguous bf16
rows) streamed at use, transposes+copies ahead of ct-ordered matmuls into 4
single-ft PSUM accumulators, per-token scale folded onto output columns;
half A's o-proj overlaps half B's tail. No collectives. Host reassembles
transposed shards.
"""
import sys
import math

sys.path.insert(0, "/opt/trn_rl_repo")

import numpy as np

B, S, H, NH = 2, 2048, 1024, 16
HD = H // NH          # 64
SHARD = 512           # query rows per core
NCORES = 8
SQRT7 = math.sqrt(7.0)
MAGIC = float(np.float32(3 * 2**22))   # fp32 RNE magic (used on tiny tiles)
MAGIC16 = 1536.0                       # 1.5 * 2^10: fp16 RNE magic

_cache = {}


def _build():
    import concourse.bass as bass
    import concourse.bacc as bacc
    import concourse.mybir as mybir
    from concourse.tile import TileContext
    from concourse.masks import make_identity

    dt = mybir.dt
    Alu = mybir.AluOpType
    Act = mybir.ActivationFunctionType
    X = mybir.AxisListType.X
    DR = mybir.MatmulPerfMode.DoubleRow

    HT_ = H // 128

    nc = bacc.Bacc("TRN2", target_bir_lowering=False, debug=False,
                   num_devices=NCORES)

    hs_in = nc.dram_tensor("hs", [S, H], dt.float32, kind="ExternalInput")
    wq8_in = nc.dram_tensor("wq8", [H, H], dt.float8e4, kind="ExternalInput")
    wk8_in = nc.dram_tensor("wk8", [H, H], dt.float8e4, kind="ExternalInput")
    wv8_in = nc.dram_tensor("wv8", [H, H], dt.float8e4, kind="ExternalInput")
    woH_in = nc.dram_tensor("woH", [HT_, 128, HT_, 128], dt.bfloat16,
                            kind="ExternalInput")
    cst_in = nc.dram_tensor("cst", [8], dt.float32, kind="ExternalInput")
    outT_out = nc.dram_tensor("outT", [H, SHARD], dt.float32, kind="ExternalOutput")

    brow = nc.dram_tensor("brow", [S], dt.float32)
    srow = nc.dram_tensor("srow", [SHARD], dt.float16)
    qTd = nc.dram_tensor("qTd", [H, SHARD], dt.float32r)

    NT = S // 128       # 16 s-tiles (all tokens)
    QT = SHARD // 128   # 4 q-tiles (own queries)
    HT = H // 128       # 8 h/f/c-tiles
    KT = S // 128       # 16 k-tiles

    IS7 = float(np.float32(1.0 / (H * SQRT7)))      # 1/(H*sqrt7)
    ES7 = float(np.float32(1e-5 / SQRT7))           # 1e-5/sqrt7

    with TileContext(nc) as tc:
        with tc.tile_pool(name="base", bufs=1) as bp, \
             tc.tile_pool(name="work", bufs=3) as wp, \
             tc.tile_pool(name="mmps", bufs=2, space="PSUM") as pmm:

            ident = bp.tile([128, 128], dt.float32)
            make_identity(nc, ident[:])
            identb = bp.tile([128, 128], dt.bfloat16)
            nc.vector.tensor_copy(identb[:], ident[:])
            identh = bp.tile([128, 128], dt.float16)
            nc.vector.tensor_copy(identh[:], ident[:])
            ones_row = bp.tile([1, 128], dt.float32)
            nc.vector.memset(ones_row[:], 1.0)
            ones16 = bp.tile([1, 128], dt.float16)
            nc.vector.memset(ones16[:], 1.0)

            cst_sb = bp.tile([1, 8], dt.float32)
            nc.sync.dma_start(out=cst_sb[:], in_=cst_in[None, :])
            ps_c = pmm.tile([128, 512], dt.float32, tag="mm")
            nc.tensor.matmul(ps_c[:, 0:8], ones_row[:], cst_sb[:], start=True, stop=True)
            cst_bc = bp.tile([128, 8], dt.float32)
            nc.vector.tensor_copy(cst_bc[:], ps_c[:, 0:8])
            AQ8 = cst_bc[:, 0:1]
            AK = cst_bc[:, 1:2]
            AV = cst_bc[:, 2:3]
            AO127 = cst_bc[:, 3:4]

            av_cols = bp.tile([128, NT], dt.float32)

            with tc.tile_pool(name="kv", bufs=1) as kvp:
                kT = kvp.tile([128, HT, S], dt.float32r)
                vres = kvp.tile([128, KT, NH, HD + 1], dt.float32r)

                # ====== merged phase 1-3: quant + projections, pipelined ====
                with tc.tile_pool(name="xq", bufs=1) as xp, \
                     tc.tile_pool(name="wts", bufs=1) as wtp, \
                     tc.tile_pool(name="scl", bufs=1) as sp, \
                     tc.tile_pool(name="qtmp", bufs=1) as qp, \
                     tc.tile_pool(name="prps", bufs=4, space="PSUM") as prps, \
                     tc.tile_pool(name="tps8", bufs=2, space="PSUM") as tpp:
                    xqT8 = xp.tile([128, HT, S], dt.float8e4)
                    wq8 = wtp.tile([128, HT, H], dt.float8e4)
                    wk8 = wtp.tile([128, HT, H], dt.float8e4)
                    wv8 = wtp.tile([128, HT, H], dt.float8e4)
                    aq_bc = sp.tile([128, SHARD], dt.float32)
                    ak_tiles = {}

                    def emit_piece(ks, p):
                        if p == 0:
                            # per-slice scale row: keys ks*512..+512
                            beta_row = wp.tile([1, 512], dt.float32, tag="brw",
                                               bufs=1)
                            nc.sync.dma_start(
                                out=beta_row[:],
                                in_=brow[None, ks * 512:(ks + 1) * 512])
                            ps_a = pmm.tile([128, 512], dt.float32, tag="mm")
                            nc.tensor.matmul(ps_a[:], ones_row[:], beta_row[:],
                                             start=True, stop=True)
                            ak_sl = sp.tile([128, 512], dt.float32,
                                            tag="aksl", bufs=2, name=f"ak{ks}")
                            ak_tiles[ks] = ak_sl
                            nc.vector.tensor_scalar(
                                out=ak_sl[:], in0=ps_a[:], scalar1=AK,
                                scalar2=None, op0=Alu.mult)
                            if ks == 0:
                                nc.vector.tensor_scalar(out=aq_bc[:], in0=ps_a[:],
                                                        scalar1=AQ8, scalar2=None,
                                                        op0=Alu.mult)
                        if ks == 0 and p >= 2:
                            # q projection (own 512 rows), 4 ft per piece
                            for ft in range(4 * (p - 2), 4 * (p - 2) + 4):
                                ps = prps.tile([128, 512], dt.float32, tag="pj")
                                for hp in range(4):
                                    nc.tensor.matmul(ps[:],
                                                     wq8[:, 2 * hp:2 * hp + 2, ft * 128:(ft + 1) * 128],
                                                     xqT8[:, 2 * hp:2 * hp + 2, 0:SHARD],
                                                     start=(hp == 0), stop=(hp == 3),
                                                     perf_mode=DR)
                                qsl = qp.tile([128, SHARD], dt.float32r, tag="qsl", bufs=1)
                                nc.vector.tensor_tensor(out=qsl[:], in0=ps[:],
                                                        in1=aq_bc[:], op=Alu.mult)
                                nc.sync.dma_start(
                                    out=qTd[ft * 128:(ft + 1) * 128, :], in_=qsl[:])
                        # k projection, 2 ft per piece
                        for ft in (2 * p, 2 * p + 1):
                            ps = prps.tile([128, 512], dt.float32, tag="pj")
                            for hp in range(4):
                                nc.tensor.matmul(ps[:],
                                                 wk8[:, 2 * hp:2 * hp + 2, ft * 128:(ft + 1) * 128],
                                                 xqT8[:, 2 * hp:2 * hp + 2, ks * 512:(ks + 1) * 512],
                                                 start=(hp == 0), stop=(hp == 3),
                                                 perf_mode=DR)
                            nc.vector.tensor_tensor(
                                out=kT[:, ft, ks * 512:(ks + 1) * 512], in0=ps[:],
                                in1=ak_tiles[ks][:], op=Alu.mult)
                        # v projection: k-tiles spread over pieces 1-3
                        for dkt in [[], [0], [1, 2], [3]][p]:
                            kt = 4 * ks + dkt
                            for fc in range(2):
                                ps = prps.tile([128, 512], dt.float32, tag="pj")
                                for hp in range(4):
                                    nc.tensor.matmul(ps[:],
                                                     xqT8[:, 2 * hp:2 * hp + 2, kt * 128:(kt + 1) * 128],
                                                     wv8[:, 2 * hp:2 * hp + 2, fc * 512:(fc + 1) * 512],
                                                     start=(hp == 0), stop=(hp == 3),
                                                     perf_mode=DR)
                                nc.scalar.activation(
                                    vres[:, kt, fc * 8:(fc + 1) * 8, 0:HD],
                                    ps[:].rearrange("p (h d) -> p h d", d=HD),
                                    Act.Copy, scale=av_cols[:, kt:kt + 1])

                    for i in range(NT):
                        hst = qp.tile([128, H], dt.float32, tag="hs", bufs=3)
                        nc.sync.dma_start(out=hst[:], in_=hs_in[i * 128:(i + 1) * 128, :])
                        # weight loads deferred behind the first hidden-state
                        # tiles: not needed until the i==3 projection burst
                        if i == 1:
                            nc.sync.dma_start(out=wk8[:], in_=wk8_in.rearrange("(a p) f -> p a f", p=128))
                        elif i == 3:
                            nc.sync.dma_start(out=wv8[:], in_=wv8_in.rearrange("(a p) f -> p a f", p=128))
                        elif i == 5:
                            nc.sync.dma_start(out=wq8[:], in_=wq8_in.rearrange("(a p) f -> p a f", p=128))
                        absj = qp.tile([128, H], dt.float8e4, tag="absj", bufs=1)
                        ssum = wp.tile([128, 1], dt.float32, tag="ssum", bufs=6)
                        nc.scalar.activation(absj[:], hst[:], Act.Abs,
                                             accum_out=ssum[:])
                        beta = wp.tile([128, 1], dt.float32, tag="beta", bufs=6)
                        nc.vector.tensor_scalar(out=beta[:], in0=ssum[:],
                                                scalar1=float(np.float32(1.0 / H)),
                                                scalar2=None, op0=Alu.mult)
                        nc.sync.dma_start(out=brow[i * 128:(i + 1) * 128], in_=beta[:, 0])
                        nc.vector.tensor_scalar(out=av_cols[:, i:i + 1], in0=beta[:],
                                                scalar1=AV, scalar2=None, op0=Alu.mult)
                        # r2s7 = sqrt7 / (beta + 1e-5) in one recip:
                        # denom2 = ssum/(H*s7) + 1e-5/s7
                        denom2 = wp.tile([128, 1], dt.float32, tag="dn2", bufs=6)
                        nc.vector.tensor_scalar(out=denom2[:], in0=ssum[:],
                                                scalar1=IS7, scalar2=ES7,
                                                op0=Alu.mult, op1=Alu.add)
                        r2s7 = wp.tile([128, 1], dt.float32, tag="r2s7", bufs=6)
                        nc.vector.reciprocal(r2s7[:], denom2[:])
                        # exact RNE to integer via the fp32 magic constant
                        # (single fp32 rounding, matches np.round); clip after
                        # (max on DVE, min on the idle Pool engine)
                        y1 = qp.tile([128, H], dt.float32, tag="y1", bufs=1)
                        nc.vector.tensor_scalar(out=y1[:], in0=hst[:],
                                                scalar1=r2s7[:],
                                                scalar2=MAGIC,
                                                op0=Alu.mult, op1=Alu.add)
                        y2 = qp.tile([128, H], dt.float16, tag="y2", bufs=2)
                        nc.vector.tensor_scalar(out=y2[:], in0=y1[:],
                                                scalar1=MAGIC,
                                                scalar2=float(np.float32(-8.0)),
                                                op0=Alu.subtract, op1=Alu.max)
                        nc.gpsimd.tensor_scalar(out=y2[:], in0=y2[:],
                                                scalar1=float(np.float32(7.0)),
                                                scalar2=None, op0=Alu.min)
                        tp8 = tpp.tile([128, H], dt.float16, tag="tp8")
                        for jt in range(HT):
                            nc.tensor.transpose(tp8[:, jt * 128:(jt + 1) * 128],
                                                y2[:, jt * 128:(jt + 1) * 128],
                                                identh[:])
                        # PSUM->SBUF convert to fp8 (GPSIMD cannot read PSUM)
                        if i % 2 == 0:
                            nc.scalar.activation(
                                xqT8[:, :, i * 128:(i + 1) * 128],
                                tp8[:].rearrange("p (a q) -> p a q", q=128),
                                Act.Copy)
                        else:
                            nc.vector.tensor_copy(
                                xqT8[:, :, i * 128:(i + 1) * 128],
                                tp8[:].rearrange("p (a q) -> p a q", q=128))

                        # projection work of slice ks is emitted in 4
                        # pieces interleaved with the NEXT 4 tiles' quant
                        # chains, so no engine queue blocks long on PE bursts
                        if i >= 4:
                            emit_piece((i // 4) - 1, i % 4)

                    for p in range(4):
                        emit_piece(3, p)

                ones_f = wp.tile([128, NH], dt.float32, tag="onesf")
                nc.vector.memset(ones_f[:], 1.0)
                ones_b = wp.tile([128, NH], dt.float32r, tag="onesb")
                nc.vector.tensor_copy(ones_b[:], ones_f[:])
                for t in range(KT):
                    nc.vector.tensor_copy(
                        vres[:, t, :, HD:HD + 1],
                        ones_b.rearrange("p (h o) -> p h o", o=1))

                # ===== phase 4+5: attention by query-halves; the tail and
                # o-projection of half A (DVE/ACT/PE mix) run concurrently
                # with attention of half B ===================================
                QW = 256
                with tc.tile_pool(name="tailp", bufs=1) as tlp:
                    nm = tlp.tile([128, QT, H], dt.bfloat16)
                    ctx = tlp.tile([128, QT, H], dt.float32)

                    def emit_tail_half(qh):
                        sts = (2 * qh, 2 * qh + 1)
                        nbs2, abs2, junks2 = [], [], []
                        for s2, st in enumerate(sts):
                            cx = ctx[:, st, :]
                            gmax = wp.tile([128, 1], dt.float32, tag=f"gm{s2}")
                            nc.vector.tensor_reduce(gmax[:], cx, axis=X, op=Alu.max,
                                                    apply_absolute_value=True)
                            gmax = gmax[:]
                            gd = wp.tile([128, 1], dt.float32, tag=f"gd{s2}")
                            nc.vector.tensor_scalar(out=gd[:], in0=gmax,
                                                    scalar1=float(np.float32(1e-5)),
                                                    scalar2=None, op0=Alu.add)
                            rg = wp.tile([128, 1], dt.float32, tag=f"rg{s2}")
                            nc.vector.reciprocal(rg[:], gd[:])
                            rg127 = wp.tile([128, 1], dt.float32, tag=f"rh{s2}")
                            nc.vector.tensor_scalar(out=rg127[:], in0=rg[:],
                                                    scalar1=float(np.float32(127.0)),
                                                    scalar2=None, op0=Alu.mult)
                            sc = wp.tile([128, 1], dt.float16, tag=f"sc{s2}")
                            nc.vector.tensor_scalar(out=sc[:], in0=gmax,
                                                    scalar1=AO127,
                                                    scalar2=None, op0=Alu.mult)
                            nc.sync.dma_start(out=srow[st * 128:(st + 1) * 128],
                                              in_=sc[:, 0])
                            # y = cx*rg*127 + 1536 -> fp16 RNE to int8 level
                            y = tlp.tile([128, H], dt.float16, tag="y", bufs=2)
                            nc.vector.tensor_scalar(out=y[:], in0=cx,
                                                    scalar1=rg127[:],
                                                    scalar2=MAGIC16,
                                                    op0=Alu.mult, op1=Alu.add)
                            nb = tlp.tile([128, H], dt.bfloat16, tag="nb", bufs=2)
                            nc.vector.tensor_scalar(out=nb[:], in0=y[:],
                                                    scalar1=MAGIC16, scalar2=None,
                                                    op0=Alu.subtract)
                            ab = tlp.tile([128, H], dt.bfloat16, tag="ab", bufs=2)
                            nc.vector.scalar_tensor_tensor(out=ab[:], in0=nb[:],
                                                           scalar=-1.0, in1=nb[:],
                                                           op0=Alu.mult, op1=Alu.max)
                            junk = tlp.tile([128, H], dt.bfloat16, tag="junk", bufs=2)
                            nbs2.append(nb); abs2.append(ab); junks2.append(junk)
                        # Real-valued bisection for the 512th-smallest |level|:
                        # invariant cnt_le(lo) < 512 <= cnt_le(hi); levels are
                        # integers in [0,127], so 7 halvings of [-0.5, 127]
                        # leave hi-lo < 1 and the threshold t* = the unique
                        # integer in (lo, hi]. Mask keeps ab > hi-1 == ab >= t*.
                        lo2 = wp.tile([128, 2], dt.float32, tag="lo2")
                        hi2 = wp.tile([128, 2], dt.float32, tag="hi2")
                        mid2 = wp.tile([128, 2], dt.float32, tag="mid2")
                        cnt2 = wp.tile([128, 2], dt.float32, tag="cnt2")
                        tk2 = wp.tile([128, 2], dt.uint32, tag="tk2")
                        nc.vector.memset(lo2[:], -0.5)
                        nc.vector.memset(hi2[:], 127.0)
                        for it in range(7):
                            nc.vector.tensor_tensor(out=mid2[:], in0=lo2[:],
                                                    in1=hi2[:], op=Alu.add)
                            nc.vector.tensor_scalar(out=mid2[:], in0=mid2[:],
                                                    scalar1=float(np.float32(0.5)),
                                                    scalar2=None, op0=Alu.mult)
                            # per-slice count on DVE: #(ab <= mid); tensor_scalar
                            # +accum gets the 4x 16-bit DVE mode (~327ns)
                            for s2 in range(2):
                                nc.vector.tensor_scalar(
                                    out=junks2[s2][:], in0=abs2[s2][:],
                                    scalar1=mid2[:, s2:s2 + 1],
                                    scalar2=None,
                                    op0=Alu.is_le, op1=Alu.add,
                                    accum_out=cnt2[:, s2:s2 + 1])
                            nc.vector.tensor_scalar(out=tk2[:], in0=cnt2[:],
                                                    scalar1=float(np.float32(512.0)),
                                                    scalar2=None, op0=Alu.is_ge)
                            nc.vector.copy_predicated(hi2[:], tk2[:], mid2[:])
                            nc.vector.tensor_scalar(out=tk2[:], in0=cnt2[:],
                                                    scalar1=float(np.float32(512.0)),
                                                    scalar2=None, op0=Alu.is_lt)
                            nc.vector.copy_predicated(lo2[:], tk2[:], mid2[:])
                        th2 = wp.tile([128, 2], dt.float32, tag="th2")
                        nc.vector.tensor_scalar(out=th2[:], in0=hi2[:],
                                                scalar1=float(np.float32(-1.0)),
                                                scalar2=None, op0=Alu.add)
                        for s2, st in enumerate(sts):
                            nc.vector.scalar_tensor_tensor(
                                out=nm[:, st, :], in0=abs2[s2][:],
                                scalar=th2[:, s2:s2 + 1], in1=nbs2[s2][:],
                                op0=Alu.is_gt, op1=Alu.mult)

                    with tc.tile_pool(name="scps", bufs=2, space="PSUM") as psc, \
                         tc.tile_pool(name="ctxps", bufs=2, space="PSUM") as pcx, \
                         tc.tile_pool(name="probs", bufs=3) as prp, \
                         tc.tile_pool(name="cwork", bufs=2) as cwp:
                        for qh in range(2):
                            qlo = qh * QW
                            for pr in range(NH // 2):
                                hA, hB = 2 * pr, 2 * pr + 1
                                qTs = cwp.tile([128, QW], dt.float32r, tag="qts")
                                nc.sync.dma_start(
                                    out=qTs[:],
                                    in_=qTd[pr * 128:(pr + 1) * 128, qlo:qlo + QW])
                                pcA = pcx.tile([HD + 1, QW], dt.float32, tag="ctx")
                                pcB = pcx.tile([HD + 1, QW], dt.float32, tag="ctx")
                                for g in range(KT // 4):
                                    psA = psc.tile([128, 1024], dt.float32, tag="sc")
                                    psB = psc.tile([128, 1024], dt.float32, tag="sc")
                                    for gi in range(4):
                                        t = 4 * g + gi
                                        ksl = kT[:, pr, t * 128:(t + 1) * 128]
                                        nc.tensor.matmul(psA[:, gi * QW:(gi + 1) * QW],
                                                         ksl[0:64, :], qTs[0:64, :],
                                                         start=True, stop=True,
                                                         tile_position=(0, 0))
                                        nc.tensor.matmul(psB[:, gi * QW:(gi + 1) * QW],
                                                         ksl[64:128, :], qTs[64:128, :],
                                                         start=True, stop=True,
                                                         tile_position=(64, 0))
                                    pbA = prp.tile([128, 1024], dt.float32r, tag="pb")
                                    pbB = prp.tile([128, 1024], dt.float32r, tag="pb")
                                    nc.scalar.activation(pbA[:], psA[:], Act.Exp)
                                    nc.scalar.activation(pbB[:], psB[:], Act.Exp)
                                    for gi in range(4):
                                        t = 4 * g + gi
                                        nc.tensor.matmul(pcA[:], vres[:, t, hA, :],
                                                         pbA[:, gi * QW:(gi + 1) * QW],
                                                         start=(t == 0), stop=(t == KT - 1))
                                        nc.tensor.matmul(pcB[:], vres[:, t, hB, :],
                                                         pbB[:, gi * QW:(gi + 1) * QW],
                                                         start=(t == 0), stop=(t == KT - 1))
                                csbA = cwp.tile([HD + 1, QW], dt.float32, tag="csbA")
                                nc.vector.tensor_copy(csbA[:], pcA[:])
                                csbB = cwp.tile([HD + 1, QW], dt.float32, tag="csbB")
                                nc.vector.tensor_copy(csbB[:], pcB[:])
                                for st2 in range(2):
                                    st = 2 * qh + st2
                                    for hx, csb in ((0, csbA), (1, csbB)):
                                        pt = pmm.tile([128, 512], dt.float32, tag="mm")
                                        nc.tensor.transpose(
                                            pt[:, 0:HD + 1],
                                            csb[:, st2 * 128:(st2 + 1) * 128],
                                            ident[0:HD + 1, 0:HD + 1])
                                        rz = wp.tile([128, 1], dt.float32)
                                        nc.vector.reciprocal(rz[:], pt[:, HD:HD + 1])
                                        nc.vector.tensor_scalar(
                                            out=ctx[:, st, (hA + hx) * HD:(hA + hx + 1) * HD],
                                            in0=pt[:, 0:HD], scalar1=rz[:],
                                            scalar2=None, op0=Alu.mult)
                            # tail of half A overlaps half B's attention;
                            # half B's tail is emitted in phase 6 after half
                            # A's o-projection so A's output DMAs go early
                            if qh == 0:
                                emit_tail_half(0)

                    # ===== phase 6: o-projection per query-half. Half A
                    # depends only on tail A (long done), so its PE work
                    # overlaps tail B's DVE/ACT bisection. ==================
                    with tc.tile_pool(name="tpbp", bufs=2, space="PSUM") as tbp, \
                         tc.tile_pool(name="opps", bufs=4, space="PSUM") as opp, \
                         tc.tile_pool(name="oph", bufs=1) as oph:
                        def emit_oproj_half(qh):
                            # per-token output scale, broadcast across rows
                            sc_row = wp.tile([1, QW], dt.float16, tag="scrow", bufs=1)
                            nc.sync.dma_start(out=sc_row[:],
                                              in_=srow[None, qh * QW:(qh + 1) * QW])
                            ps_s = pmm.tile([128, 512], dt.float32, tag="mm")
                            nc.tensor.matmul(ps_s[:, 0:QW], ones16[:], sc_row[:],
                                             start=True, stop=True)
                            sc_bc = oph.tile([128, QW], dt.float32, tag="scbc", bufs=2)
                            nc.vector.tensor_copy(sc_bc[:], ps_s[:, 0:QW])
                            rhsT = oph.tile([128, HT, QW], dt.bfloat16, tag="rhsT",
                                            bufs=1)
                            # all transposes+copies first, then matmuls, so PE
                            # is never queue-blocked behind a copy; two waves
                            # of 4 ft so each accumulation chain owns a bank
                            for ct in range(HT):
                                tpb = tbp.tile([128, QW], dt.bfloat16, tag="tpb")
                                for st2, st in enumerate((2 * qh, 2 * qh + 1)):
                                    nc.tensor.transpose(
                                        tpb[:, st2 * 128:(st2 + 1) * 128],
                                        nm[:, st, ct * 128:(ct + 1) * 128],
                                        identb[:])
                                nc.scalar.activation(rhsT[:, ct, :], tpb[:],
                                                     Act.Copy)
                            for wv in range(2):
                                pss = [opp.tile([128, QW], dt.float32, tag="op",
                                                name=f"op{qh}_{wv}_{fp}")
                                       for fp in range(4)]
                                wsls = []
                                for fp in range(4):
                                    ft = 4 * wv + fp
                                    wsl = oph.tile([128, HT, 128], dt.bfloat16,
                                                   tag="wsl", bufs=7)
                                    nc.sync.dma_start(out=wsl[:], in_=woH_in[ft])
                                    wsls.append(wsl)
                                for ct in range(HT):
                                    for fp in range(4):
                                        nc.tensor.matmul(pss[fp][:],
                                                         wsls[fp][:, ct, :],
                                                         rhsT[:, ct, :], start=(ct == 0),
                                                         stop=(ct == HT - 1))
                                for fp in range(4):
                                    ft = 4 * wv + fp
                                    ot = oph.tile([128, QW], dt.float32, tag="ot", bufs=4)
                                    nc.vector.tensor_tensor(out=ot[:], in0=pss[fp][:],
                                                            in1=sc_bc[:], op=Alu.mult)
                                    nc.sync.dma_start(
                                        out=outT_out[ft * 128:(ft + 1) * 128,
                                                     qh * QW:(qh + 1) * QW],
                                        in_=ot[:])

                        emit_oproj_half(0)
                        emit_tail_half(1)
                        emit_oproj_half(1)

    nc.compile()
    return nc


def kernel(hidden_states, Wq, Wk, Wv, Wo, sq, sk, sv, so):
    import jax
    import jax.numpy as jnp
    from concourse.bass_utils import run_bass_kernel_spmd
    import ml_dtypes

    cpu = jax.devices("cpu")[0]

    def wquant(W, s):
        with jax.default_device(cpu):
            W32 = np.asarray(W, np.float32)
            w_mean = jnp.mean(jnp.abs(jnp.asarray(W32)))
            w_q = jnp.clip(jnp.round(jnp.asarray(W32) / (w_mean + 1e-5)), -1.0, 1.0)
            return np.asarray(w_q, np.float32), np.float32(np.float32(w_mean) * np.float32(s))

    hidden_states = np.ascontiguousarray(np.asarray(hidden_states, np.float32))
    wq_q, aq = wquant(Wq, np.asarray(sq).reshape(-1)[0])
    wk_q, ak = wquant(Wk, np.asarray(sk).reshape(-1)[0])
    wv_q, av = wquant(Wv, np.asarray(sv).reshape(-1)[0])
    wo_q, ao = wquant(Wo, np.asarray(so).reshape(-1)[0])

    wq8 = np.ascontiguousarray(wq_q.T).astype(ml_dtypes.float8_e4m3)
    wk8 = np.ascontiguousarray(wk_q.T).astype(ml_dtypes.float8_e4m3)
    wv8 = np.ascontiguousarray(wv_q.T).astype(ml_dtypes.float8_e4m3)
    woT_mat = np.ascontiguousarray(wo_q.T)
    woH = np.ascontiguousarray(
        woT_mat.reshape(H // 128, 128, H // 128, 128).transpose(2, 1, 0, 3)
    ).astype(ml_dtypes.bfloat16)

    cst = np.zeros(8, np.float32)
    cst[0] = np.float32(aq / np.float32(math.sqrt(HD)))
    cst[1] = ak
    cst[2] = av
    cst[3] = np.float32(ao / np.float32(127.0))

    if "nc" not in _cache:
        _cache["nc"] = _build()
    nc = _cache["nc"]

    in_maps = []
    for c in range(NCORES):
        b, j = c // 4, c % 4
        hs_rot = np.ascontiguousarray(np.roll(hidden_states[b], -j * SHARD, axis=0))
        in_maps.append({
            "hs": hs_rot,
            "wq8": wq8, "wk8": wk8, "wv8": wv8, "woH": woH, "cst": cst,
        })

    _cache["last_in_maps"] = in_maps
    res = run_bass_kernel_spmd(nc, in_maps, list(range(NCORES)))
    _cache["last_res"] = res
    out = np.empty((B, S, H), np.float32)
    for c in range(NCORES):
        b, j = c // 4, c % 4
        out[b, j * SHARD:(j + 1) * SHARD, :] = res.results[c]["outT"].T
    return out
